# revision 1
# baseline (speedup 1.0000x reference)
"""CondTransport kernel for Trainium2 (8 NeuronCores, row-parallel).

Math: Z = Y_mean + Y_var + k_mean @ V_mean + k_var @ V_var, where
k(X, X) = exp(-||x_i - x_j||^2 / (2 l^2)) are 8192x8192 RBF Gram matrices
over X_mean = [X_mu, Y_mean+Y_var] (96-d, l=7) and
X_var = [X_mu, 0.01*flip(Y_eta), Y_mean+Y_var] (160-d, l=9).

Factorization used on device:
  k[i,j] = e_i * T[j,i],   T[j,i] = exp(G_ij / l^2 - rn_j / (2 l^2)),
  G = X X^T,  rn = row norms,  e_i = exp(-rn_i / (2 l^2)).
So  Z_col[i] = e_i * sum_j T[j,i] V[j]  — one exp-Gram GEMM pipeline with
the row factor folded into ScalarE's per-partition activation bias and the
e_i factor applied on the host epilogue.

Sharding: rows i are split 1024-per-core (8 cores); each core holds the
full X (column side), computes G tiles [128 j x 1024 i] (PE fp16 matmul —
fp16 products are exact in the fp32 PSUM accumulation, and the host
derives the row norms from the quantized features so sq stays
consistent), exps them twice straight out of PSUM (ScalarE, scale 1/49
for mean and 1/81 for var, fp16 output), and accumulates both
Z^T [64, 1024] in PSUM over all 64 j-tiles (fp16 E tiles x fp16 V
weights). Measured vs the fp32 reference: ~7e-5 max relative error.

The 0.01-scaled flip(Y_eta) features shift the var exponent by under
3.2e-4 (typ 8e-5) — below the fp16 Gram quantization noise — so the var
Gram reuses the mean Gram (USE_F=True restores the exact rank-64
correction at the cost of PSUM double-buffering).
"""

import numpy as np

N = 8192
DX = 32
DY = 64
NCORES = 8
ROWS = N // NCORES          # 1024 rows per core
NJT = N // 128              # 64 j-tiles
DM = DX + DY                # 96 mean features
L_MEAN = 7.0
L_VAR = 9.0
VAR_EPS = 0.01

USE_F = False  # exact rank-64 var-feature correction (see module docstring)

_CACHE = {}


def _build_nc():
    import concourse.mybir as mybir
    import concourse.tile as tile
    from concourse import bacc

    f32 = mybir.dt.float32
    f32r = mybir.dt.float32r
    f16 = mybir.dt.float16
    Exp = mybir.ActivationFunctionType.Exp

    nc = bacc.Bacc(None, target_bir_lowering=False)

    xmT_all = nc.declare_dram_parameter("xmT_all", [DM, N], f16, isOutput=False)
    xmT_own = nc.declare_dram_parameter("xmT_own", [DM, ROWS], f16, isOutput=False)
    if USE_F:
        fT_all = nc.declare_dram_parameter("fT_all", [DY, N], f16, isOutput=False)
        fT_own = nc.declare_dram_parameter("fT_own", [DY, ROWS], f16, isOutput=False)
    vm = nc.declare_dram_parameter("vm", [128, NJT * DY], f16, isOutput=False)
    vv = nc.declare_dram_parameter("vv", [128, NJT * DY], f16, isOutput=False)
    bias_m = nc.declare_dram_parameter("bias_m", [128, NJT], f32, isOutput=False)
    bias_v = nc.declare_dram_parameter("bias_v", [128, NJT], f32, isOutput=False)
    zT = nc.declare_dram_parameter("zT", [128, ROWS], f32, isOutput=True)

    inv2lm = float(1.0 / (L_MEAN * L_MEAN))
    inv2lv = float(1.0 / (L_VAR * L_VAR))

    with tile.TileContext(nc) as tc:
        with (
            tc.tile_pool(name="data", bufs=1) as data,
            tc.tile_pool(name="etiles", bufs=4) as etiles,
            tc.tile_pool(name="psg", bufs=2, space="PSUM") as psg,
            tc.tile_pool(name="psz", bufs=1, space="PSUM") as psz,
        ):
            sxm_own = data.tile([DM, ROWS], f16)
            sbias_m = data.tile([128, NJT], f32)
            sbias_v = data.tile([128, NJT], f32)
            sxm = data.tile([DM, N], f16)
            svm = data.tile([128, NJT * DY], f16)
            svv = data.tile([128, NJT * DY], f16)

            # Critical path to the first ACT: own rows (rhs) + j-block 0 of
            # X^T (weights). Issue those first, in small pieces, then stream
            # the rest column-chunked across separate engine queues so issue
            # serialization doesn't delay the first matmuls.
            nc.sync.dma_start(out=sxm_own[:, 0 : ROWS // 2], in_=xmT_own[:, 0 : ROWS // 2])
            nc.sync.dma_start(out=sxm[:, 0:128], in_=xmT_all[:, 0:128])
            nc.sync.dma_start(
                out=sxm_own[:, ROWS // 2 : ROWS], in_=xmT_own[:, ROWS // 2 : ROWS]
            )
            nc.scalar.dma_start(out=sbias_m, in_=bias_m[:, :])
            nc.scalar.dma_start(out=sbias_v, in_=bias_v[:, :])
            if USE_F:
                sf_own = data.tile([DY, ROWS], f16)
                sf = data.tile([DY, N], f16)
                nc.sync.dma_start(out=sf_own, in_=fT_own[:, :])
            # Column-chunked loads so the first j-tiles' matmuls can start
            # before the whole working set lands.
            CH = N // 8
            for k in range(8):
                cs = slice(k * CH + (128 if k == 0 else 0), (k + 1) * CH)
                vs = slice(k * (NJT * DY) // 8, (k + 1) * (NJT * DY) // 8)
                # V chunk k feeds the Z matmuls of the same j-tiles as X
                # chunk k; issue it first so Z(0) isn't stuck behind the
                # bulk X transfer on the single sync hardware queue.
                nc.sync.dma_start(out=svm[:, vs], in_=vm[:, vs])
                nc.sync.dma_start(out=svv[:, vs], in_=vv[:, vs])
                nc.sync.dma_start(out=sxm[:, cs], in_=xmT_all[:, cs])
                if USE_F:
                    nc.sync.dma_start(out=sf[:, cs], in_=fT_all[:, cs])

            pzm = psz.tile([64, ROWS], f32)
            pzv = psz.tile([64, ROWS], f32)

            H = ROWS // 2  # 512-wide halves (PSUM bank / fp32 moving-op limit)

            # Warm-up matmuls on zero data while the first DMAs land: keeps
            # the PE HAM activity window busy so the real matmuls start at
            # 2.4 GHz instead of the cold 1.2 GHz. Overwritten by the real
            # start=True accumulation below.
            warm = data.tile([DM, H], f16)
            nc.vector.memset(warm, 0.0)
            for w in range(2):
                nc.tensor.matmul(
                    (pzm if w % 2 == 0 else pzv)[:, 0:H],
                    warm[:, 0:64], warm[:, :], start=True, stop=True,
                )

            def emit_gram(jt):
                """G(jt) tiles [128 j x ROWS i] into rotating PSUM slots.

                Returns (mean_gram, var_gram); without USE_F they are the
                same tile (the f-feature correction is dropped)."""
                jb = slice(jt * 128, (jt + 1) * 128)
                gb = 1 if USE_F else 2
                pg = psg.tile([128, ROWS], f32, tag="pgm", bufs=gb, name=f"pgm{jt}")
                if USE_F:
                    pgv = psg.tile(
                        [128, ROWS], f32, tag="pgv", bufs=1, name=f"pgv{jt}"
                    )
                for h in range(2):
                    hs = slice(h * H, (h + 1) * H)
                    nc.tensor.matmul(
                        pg[:, hs], sxm[:, jb], sxm_own[:, hs], start=True, stop=True
                    )
                    if USE_F:
                        nc.tensor.matmul(
                            pgv[:, hs], sxm[:, jb], sxm_own[:, hs],
                            start=True, stop=False,
                        )
                        nc.tensor.matmul(
                            pgv[:, hs], sf[:, jb], sf_own[:, hs],
                            start=False, stop=True,
                        )
                return pg, (pgv if USE_F else pg)

            # Main loop: G(jt) -> two exps -> Z accumulation. With the
            # double-buffered G slots the Tile scheduler hoists G(jt+1)
            # into ACT_v(jt)'s shadow, keeping ScalarE's exp stream gapless
            # (~0.5 us total gaps measured over 128 ACTIVATEs).
            grams = [emit_gram(0), emit_gram(1)]
            for jt in range(NJT):
                st = jt == 0
                sp = jt == NJT - 1
                pg, pgv = grams[jt]

                em = etiles.tile([128, ROWS], f16, tag="em", name=f"em{jt}")
                ev = etiles.tile([128, ROWS], f16, tag="ev", name=f"ev{jt}")
                nc.scalar.activation(
                    em[:, :], pg[:, :], Exp,
                    bias=sbias_m[:, jt : jt + 1], scale=inv2lm,
                )
                nc.scalar.activation(
                    ev[:, :], pgv[:, :], Exp,
                    bias=sbias_v[:, jt : jt + 1], scale=inv2lv,
                )

                vb = slice(jt * DY, (jt + 1) * DY)
                for h in range(2):
                    hs = slice(h * H, (h + 1) * H)
                    nc.tensor.matmul(
                        pzm[:, hs], svm[:, vb], em[:, hs], start=st, stop=sp
                    )
                    nc.tensor.matmul(
                        pzv[:, hs], svv[:, vb], ev[:, hs], start=st, stop=sp
                    )
                if jt + 2 < NJT:
                    grams.append(emit_gram(jt + 2))

            szT = data.tile([128, ROWS], f32)
            nc.scalar.copy(szT[0:64, :], pzm[:, :])
            nc.vector.tensor_copy(szT[64:128, :], pzv[:, :])
            nc.sync.dma_start(out=zT[0:64, :], in_=szT[0:64, :])
            nc.sync.dma_start(out=zT[64:128, :], in_=szT[64:128, :])

    nc.finalize()
    return nc


def _get_nc():
    if "nc" not in _CACHE:
        _CACHE["nc"] = _build_nc()
    return _CACHE["nc"]


def prep_inputs(X_mu, Y_eta, Y_mean, Y_var, V_mean, V_var):
    """Host-side prep: layouts, norms, biases. Returns (in_maps, e_m, e_v, ymv)."""
    X_mu, Y_eta, Y_mean, Y_var, V_mean, V_var = (
        np.asarray(a, dtype=np.float32)
        for a in (X_mu, Y_eta, Y_mean, Y_var, V_mean, V_var)
    )
    ymv = (Y_mean.astype(np.float64) + Y_var.astype(np.float64)).astype(np.float32)
    # fp16 features: PE products of fp16 inputs are exact in the fp32 PSUM
    # accumulation, so deriving the row norms from the QUANTIZED features
    # keeps sq = rn_i + rn_j - 2G consistent to ~1e-4.
    Xm = np.concatenate([X_mu, ymv], axis=1).astype(np.float32).astype(np.float16)
    f = (VAR_EPS * Y_eta[::-1].astype(np.float64)).astype(np.float16)  # [N, 64]

    rn_m = np.sum(Xm.astype(np.float64) ** 2, axis=1)                # [N]
    rn_v = rn_m + (np.sum(f.astype(np.float64) ** 2, axis=1) if USE_F else 0.0)

    bias_m = (-rn_m / (2.0 * L_MEAN * L_MEAN)).astype(np.float32)
    bias_v = (-rn_v / (2.0 * L_VAR * L_VAR)).astype(np.float32)
    # [N] -> [128, NJT] with element (p, t) = row t*128+p
    bias_m_sb = np.ascontiguousarray(bias_m.reshape(NJT, 128).T)
    bias_v_sb = np.ascontiguousarray(bias_v.reshape(NJT, 128).T)

    e_m = np.exp(-rn_m / (2.0 * L_MEAN * L_MEAN))                    # fp64 [N]
    e_v = np.exp(-rn_v / (2.0 * L_VAR * L_VAR))

    xmT = np.ascontiguousarray(Xm.T)                                 # [96, N]
    fT = np.ascontiguousarray(f.T)                                   # [64, N]
    vm_sb = np.ascontiguousarray(
        V_mean.reshape(NJT, 128, DY).transpose(1, 0, 2).reshape(128, NJT * DY)
    ).astype(np.float16)
    vv_sb = np.ascontiguousarray(
        V_var.reshape(NJT, 128, DY).transpose(1, 0, 2).reshape(128, NJT * DY)
    ).astype(np.float16)

    in_maps = []
    for c in range(NCORES):
        rs = slice(c * ROWS, (c + 1) * ROWS)
        m = dict(
            xmT_all=xmT,
            xmT_own=np.ascontiguousarray(Xm[rs].T),
            vm=vm_sb,
            vv=vv_sb,
            bias_m=bias_m_sb,
            bias_v=bias_v_sb,
        )
        if USE_F:
            m["fT_all"] = fT
            m["fT_own"] = np.ascontiguousarray(f[rs].T)
        in_maps.append(m)
    return in_maps, e_m, e_v, ymv


def postprocess(results, e_m, e_v, ymv):
    """Gather per-core z^T outputs and apply the e_i row factors + Y terms."""
    out = ymv.astype(np.float64).copy()
    for c in range(NCORES):
        rs = slice(c * ROWS, (c + 1) * ROWS)
        zT = results[c]["zT"].astype(np.float64)  # [128, ROWS]
        out[rs] += e_m[rs, None] * zT[0:64].T
        out[rs] += e_v[rs, None] * zT[64:128].T
    return out.astype(np.float32)


def kernel(X_mu, Y_eta, Y_mean, Y_var, V_mean, V_var):
    from concourse.bass_utils import run_bass_kernel_spmd

    nc = _get_nc()
    in_maps, e_m, e_v, ymv = prep_inputs(X_mu, Y_eta, Y_mean, Y_var, V_mean, V_var)
    res = run_bass_kernel_spmd(nc, in_maps, core_ids=list(range(NCORES)))
    return postprocess(res.results, e_m, e_v, ymv)



# revision 3
# speedup vs baseline: 1.1447x; 1.1447x over previous
"""CondTransport kernel for Trainium2 (8 NeuronCores, row-parallel).

Math: Z = Y_mean + Y_var + k_mean @ V_mean + k_var @ V_var, where
k(X, X) = exp(-||x_i - x_j||^2 / (2 l^2)) are 8192x8192 RBF Gram matrices
over X_mean = [X_mu, Y_mean+Y_var] (96-d, l=7) and
X_var = [X_mu, 0.01*flip(Y_eta), Y_mean+Y_var] (160-d, l=9).

Factorization: k[i,j] = e_i * e_j * exp(G_ij / l^2), G = X X^T, with the
e_j column factor folded into the V weights on the host (V'_j = e_j V_j)
and the e_i row factor applied in the host epilogue. This leaves the
device activations with compile-time-constant scale/bias only.

The 0.01-scaled flip(Y_eta) features of X_var shift the var exponent by
under 3.2e-4 - below the fp16 Gram quantization noise - so the var Gram
reuses the mean Gram.

Per j-tile the [128 x 1024] Gram needs TWO elementwise exps (mean, var)
- 2*64 tiles of ScalarE ACTIVATE was the baseline bottleneck (~143 us,
94% busy). Here the exp work is split across two engines:
  * ScalarE: exact exp for the var stream, exp(G' * kv), kv immediate.
  * VectorE: Schraudolph bit-trick exp for the mean stream - one
    tensor_scalar op computing int16(round((G' + b0) max 0)) whose bits
    ARE the fp16 approximation of exp(G/49). The Schraudolph scale
    (1024*log2e/49) is pre-folded into the features on the host, so
    G' = s*G comes out of the PE directly. Max rel err ~3% on k_mean,
    which averages out across the 8192-column GEMM reduction (measured
    end-to-end error ~1e-3, gate is 2e-2).
Both engines run ~1.15-1.25 us per tile, fully overlapped with the PE
Gram+Z matmul stream (~1.3 us/tile).

Sharding: rows i split 1024-per-core (8 cores); each core holds full X
(column side), computes G' tiles [128 j x 1024 i] (fp16 PE matmul - fp16
products are exact in fp32 PSUM, and the host derives row norms from the
quantized features so sq stays consistent), produces em (DVE) and ev
(ScalarE) fp16 tiles, and accumulates both Z^T [64, 1024] in PSUM over
all 64 j-tiles.
"""

import numpy as np

N = 8192
DX = 32
DY = 64
NCORES = 8
ROWS = N // NCORES          # 1024 rows per core
NJT = N // 128              # 64 j-tiles
DM = DX + DY                # 96 mean features
L_MEAN = 7.0
L_VAR = 9.0
VAR_EPS = 0.01

LOG2E = 1.4426950408889634
SCHRAUD_C = -44.75          # rel-err-balancing shift for round-to-nearest
S_SCALE = 1024.0 * LOG2E / (L_MEAN * L_MEAN)   # G' = S_SCALE * G
B0 = 15360.0 + SCHRAUD_C                       # fp16 exponent-bias term
KV = 1.0 / (L_VAR * L_VAR * S_SCALE)           # ACT scale: G'*KV = G/81

_CACHE = {}


def _build_nc():
    import concourse.mybir as mybir
    import concourse.tile as tile
    from concourse import bacc

    f32 = mybir.dt.float32
    f16 = mybir.dt.float16
    i16 = mybir.dt.int16
    Exp = mybir.ActivationFunctionType.Exp
    Add = mybir.AluOpType.add
    Max = mybir.AluOpType.max

    nc = bacc.Bacc(None, target_bir_lowering=False)

    xmT_all = nc.declare_dram_parameter("xmT_all", [DM, N], f16, isOutput=False)
    xmT_own = nc.declare_dram_parameter("xmT_own", [DM, ROWS], f16, isOutput=False)
    vm = nc.declare_dram_parameter("vm", [128, NJT * DY], f16, isOutput=False)
    vv = nc.declare_dram_parameter("vv", [128, NJT * DY], f16, isOutput=False)
    zT = nc.declare_dram_parameter("zT", [128, ROWS], f32, isOutput=True)

    with tile.TileContext(nc) as tc:
        with (
            tc.tile_pool(name="data", bufs=1) as data,
            tc.tile_pool(name="etiles", bufs=4) as etiles,
            tc.tile_pool(name="psg", bufs=2, space="PSUM") as psg,
            tc.tile_pool(name="psz", bufs=1, space="PSUM") as psz,
        ):
            sxm_own = data.tile([DM, ROWS], f16)
            sxm = data.tile([DM, N], f16)
            svm = data.tile([128, NJT * DY], f16)
            svv = data.tile([128, NJT * DY], f16)

            # Critical path to the first matmuls: own rows (rhs) + j-block 0
            # of X^T (weights). Issue those first, then stream the bulk
            # column-chunked so the first j-tiles' matmuls can start before
            # the whole working set lands. All on the sync queue: DMA_DIRECT2D
            # occupies the issuing engine's sequencer for the transfer, so
            # the scalar queue must stay clear for ACTIVATEs.
            nc.sync.dma_start(out=sxm_own[:, 0 : ROWS // 2], in_=xmT_own[:, 0 : ROWS // 2])
            nc.sync.dma_start(out=sxm[:, 0:128], in_=xmT_all[:, 0:128])
            nc.sync.dma_start(
                out=sxm_own[:, ROWS // 2 : ROWS], in_=xmT_own[:, ROWS // 2 : ROWS]
            )
            CH = N // 8
            for k in range(8):
                cs = slice(k * CH + (128 if k == 0 else 0), (k + 1) * CH)
                vs = slice(k * (NJT * DY) // 8, (k + 1) * (NJT * DY) // 8)
                nc.sync.dma_start(out=svm[:, vs], in_=vm[:, vs])
                nc.sync.dma_start(out=svv[:, vs], in_=vv[:, vs])
                nc.sync.dma_start(out=sxm[:, cs], in_=xmT_all[:, cs])

            pzm = psz.tile([64, ROWS], f32)
            pzv = psz.tile([64, ROWS], f32)

            H = ROWS // 2  # 512-wide halves (PSUM bank / fp32 moving-op limit)

            # Warm-up matmuls on zero data while the first DMAs land: keeps
            # the PE HAM activity window busy so the real matmuls start at
            # 2.4 GHz instead of the cold 1.2 GHz. Overwritten by the real
            # start=True accumulation below.
            warm = data.tile([DM, H], f16)
            nc.vector.memset(warm, 0.0)
            for w in range(2):
                nc.tensor.matmul(
                    (pzm if w % 2 == 0 else pzv)[:, 0:H],
                    warm[:, 0:64], warm[:, :], start=True, stop=True,
                )

            def emit_gram(jt):
                """G'(jt) tile [128 j x ROWS i] into a rotating PSUM slot."""
                jb = slice(jt * 128, (jt + 1) * 128)
                pg = psg.tile([128, ROWS], f32, tag="pgm", bufs=2, name=f"pgm{jt}")
                for h in range(2):
                    hs = slice(h * H, (h + 1) * H)
                    nc.tensor.matmul(
                        pg[:, hs], sxm[:, jb], sxm_own[:, hs], start=True, stop=True
                    )
                return pg

            # Main loop: G'(jt) -> {DVE schraudolph-exp, ACT exact-exp} ->
            # Z accumulation. Double-buffered G slots let the scheduler
            # hoist G(jt+1) matmuls into the activations' shadow.
            grams = [emit_gram(0), emit_gram(1)]
            for jt in range(NJT):
                st = jt == 0
                sp = jt == NJT - 1
                pg = grams[jt]

                em = etiles.tile([128, ROWS], f16, tag="em", name=f"em{jt}")
                ev = etiles.tile([128, ROWS], f16, tag="ev", name=f"ev{jt}")
                # em bits = int16(round((G' + B0) max 0)) == fp16 exp(G/49)
                nc.vector.tensor_scalar(em.bitcast(i16), pg[:, :], B0, 0.0, Add, Max)
                nc.scalar.activation(ev[:, :], pg[:, :], Exp, bias=0.0, scale=KV)

                vb = slice(jt * DY, (jt + 1) * DY)
                for h in range(2):
                    hs = slice(h * H, (h + 1) * H)
                    nc.tensor.matmul(
                        pzm[:, hs], svm[:, vb], em[:, hs], start=st, stop=sp
                    )
                    nc.tensor.matmul(
                        pzv[:, hs], svv[:, vb], ev[:, hs], start=st, stop=sp
                    )
                if jt + 2 < NJT:
                    grams.append(emit_gram(jt + 2))

            szT = data.tile([128, ROWS], f32)
            nc.scalar.copy(szT[0:64, :], pzm[:, :])
            nc.vector.tensor_copy(szT[64:128, :], pzv[:, :])
            nc.sync.dma_start(out=zT[0:64, :], in_=szT[0:64, :])
            nc.sync.dma_start(out=zT[64:128, :], in_=szT[64:128, :])

    nc.finalize()
    return nc


def _get_nc():
    if "nc" not in _CACHE:
        _CACHE["nc"] = _build_nc()
    return _CACHE["nc"]


def prep_inputs(X_mu, Y_eta, Y_mean, Y_var, V_mean, V_var):
    """Host-side prep: layouts, norms, folded V weights.

    Returns (in_maps, e_m, e_v, ymv)."""
    X_mu, Y_eta, Y_mean, Y_var, V_mean, V_var = (
        np.asarray(a, dtype=np.float32)
        for a in (X_mu, Y_eta, Y_mean, Y_var, V_mean, V_var)
    )
    ymv = (Y_mean.astype(np.float64) + Y_var.astype(np.float64)).astype(np.float32)
    # fp16 features pre-scaled by sqrt(S_SCALE) so the PE emits
    # G' = S_SCALE * G directly (the Schraudolph mantissa scale). fp16
    # products are exact in the fp32 PSUM accumulation, so deriving the
    # row norms from the QUANTIZED features keeps sq consistent.
    sq_s = float(np.sqrt(S_SCALE))
    Xm = np.concatenate([X_mu, ymv], axis=1).astype(np.float32)
    Xq = (Xm * sq_s).astype(np.float16)                              # [N, 96]

    rn = np.sum(Xq.astype(np.float64) ** 2, axis=1)                  # scaled units
    # guard: Schraudolph int16 must stay in (0, 32767):
    # i16 = G' + B0 with |G'| <= max rn
    assert rn.max() + B0 < 32200.0 and B0 - rn.max() > 500.0

    # k = e_i * e_j * exp(G/l^2); exponents in scaled units:
    # G/49 = G'/(1024*log2e), G/81 = G'*KV
    e_m = np.exp(-rn / (2.0 * 1024.0 * LOG2E))                       # fp64 [N]
    e_v = np.exp(-rn * (KV / 2.0))

    xmT = np.ascontiguousarray(Xq.T)                                 # [96, N]
    vm_f = e_m[:, None] * V_mean.astype(np.float64)                  # fold e_j
    vv_f = e_v[:, None] * V_var.astype(np.float64)
    vm_sb = np.ascontiguousarray(
        vm_f.reshape(NJT, 128, DY).transpose(1, 0, 2).reshape(128, NJT * DY)
    ).astype(np.float16)
    vv_sb = np.ascontiguousarray(
        vv_f.reshape(NJT, 128, DY).transpose(1, 0, 2).reshape(128, NJT * DY)
    ).astype(np.float16)

    in_maps = []
    for c in range(NCORES):
        rs = slice(c * ROWS, (c + 1) * ROWS)
        in_maps.append(dict(
            xmT_all=xmT,
            xmT_own=np.ascontiguousarray(Xq[rs].T),
            vm=vm_sb,
            vv=vv_sb,
        ))
    return in_maps, e_m, e_v, ymv


def postprocess(results, e_m, e_v, ymv):
    """Gather per-core z^T outputs and apply the e_i row factors + Y terms."""
    out = ymv.astype(np.float64).copy()
    for c in range(NCORES):
        rs = slice(c * ROWS, (c + 1) * ROWS)
        zT = results[c]["zT"].astype(np.float64)  # [128, ROWS]
        out[rs] += e_m[rs, None] * zT[0:64].T
        out[rs] += e_v[rs, None] * zT[64:128].T
    return out.astype(np.float32)


def kernel(X_mu, Y_eta, Y_mean, Y_var, V_mean, V_var):
    from concourse.bass_utils import run_bass_kernel_spmd

    nc = _get_nc()
    in_maps, e_m, e_v, ymv = prep_inputs(X_mu, Y_eta, Y_mean, Y_var, V_mean, V_var)
    res = run_bass_kernel_spmd(nc, in_maps, core_ids=list(range(NCORES)))
    return postprocess(res.results, e_m, e_v, ymv)


# revision 5
# speedup vs baseline: 1.5786x; 1.3791x over previous
"""CondTransport kernel for Trainium2 (8 NeuronCores, row-parallel).

Math: Z = Y_mean + Y_var + k_mean @ V_mean + k_var @ V_var, where
k(X, X) = exp(-||x_i - x_j||^2 / (2 l^2)) are 8192x8192 RBF Gram matrices
over X_mean = [X_mu, Y_mean+Y_var] (96-d, l=7) and
X_var = [X_mu, 0.01*flip(Y_eta), Y_mean+Y_var] (160-d, l=9).

Factorization: k[i,j] = e_i * e_j * exp(G_ij / l^2), G = X X^T, with the
e_j column factor folded into the V weights on the host (V'_j = e_j V_j)
and the e_i row factor applied in the host epilogue. This leaves the
device activations with compile-time-constant scale/bias only.

The 0.01-scaled flip(Y_eta) features of X_var shift the var exponent by
under 3.2e-4 - below the fp16 Gram quantization noise - so the var Gram
reuses the mean Gram.

Per j-tile the [128 x 1024] Gram needs TWO elementwise exps (mean, var)
- 2*64 tiles of ScalarE ACTIVATE was the baseline bottleneck (~143 us,
94% busy). Here the exp work is split across two engines:
  * ScalarE: exact exp for the var stream, exp(G' * kv), kv immediate.
  * VectorE: Schraudolph bit-trick exp for the mean stream - one
    tensor_scalar op computing int16(round((G' + b0) max 0)) whose bits
    ARE the fp16 approximation of exp(G/49). The Schraudolph scale
    (1024*log2e/49) is pre-folded into the features on the host, so
    G' = s*G comes out of the PE directly. Max rel err ~3% on k_mean,
    which averages out across the 8192-column GEMM reduction (measured
    end-to-end error ~1e-3, gate is 2e-2).
Both engines run ~1.15-1.25 us per tile, fully overlapped with the PE
Gram+Z matmul stream (~1.3 us/tile).

Sharding: rows i split 1024-per-core (8 cores); each core holds full X
(column side), computes G' tiles [128 j x 1024 i] (fp16 PE matmul - fp16
products are exact in fp32 PSUM, and the host derives row norms from the
quantized features so sq stays consistent), produces em (DVE) and ev
(ScalarE) fp16 tiles, and accumulates both Z^T [64, 1024] in PSUM over
all 64 j-tiles.
"""

import numpy as np

N = 8192
DX = 32
DY = 64
NCORES = 8
ROWS = N // NCORES          # 1024 rows per core
NJT = N // 128              # 64 j-tiles
DM = DX + DY                # 96 mean features
L_MEAN = 7.0
L_VAR = 9.0
VAR_EPS = 0.01

LOG2E = 1.4426950408889634
SCHRAUD_C = -44.75          # rel-err-balancing shift for round-to-nearest
S_SCALE = 1024.0 * LOG2E / (L_MEAN * L_MEAN)   # G' = S_SCALE * G
B0 = 15360.0 + SCHRAUD_C                       # fp16 exponent-bias term
KV = 1.0 / (L_VAR * L_VAR * S_SCALE)           # ACT scale: G'*KV = G/81

_CACHE = {}


def _build_nc():
    import concourse.mybir as mybir
    import concourse.tile as tile
    from concourse import bacc

    f32 = mybir.dt.float32
    f16 = mybir.dt.float16
    i16 = mybir.dt.int16
    Exp = mybir.ActivationFunctionType.Exp
    Add = mybir.AluOpType.add
    Max = mybir.AluOpType.max

    nc = bacc.Bacc(None, target_bir_lowering=False)

    xmT_all = nc.declare_dram_parameter("xmT_all", [DM, N], f16, isOutput=False)
    xmT_own = nc.declare_dram_parameter("xmT_own", [DM, ROWS], f16, isOutput=False)
    vm = nc.declare_dram_parameter("vm", [128, NJT * DY], f16, isOutput=False)
    vv = nc.declare_dram_parameter("vv", [128, NJT * DY], f16, isOutput=False)
    zT = nc.declare_dram_parameter("zT", [128, ROWS], f32, isOutput=True)

    with tile.TileContext(nc) as tc:
        with (
            tc.tile_pool(name="data", bufs=1) as data,
            tc.tile_pool(name="etiles", bufs=4) as etiles,
            tc.tile_pool(name="psg", bufs=2, space="PSUM") as psg,
            tc.tile_pool(name="psz", bufs=1, space="PSUM") as psz,
        ):
            sxm_own = data.tile([DM, ROWS], f16)
            sxm = data.tile([DM, N], f16)
            svm = data.tile([128, NJT * DY], f16)
            svv = data.tile([128, NJT * DY], f16)

            # Critical path to the first matmuls: own rows (rhs) + j-block 0
            # of X^T (weights). Issue those first, then stream the bulk
            # column-chunked so the first j-tiles' matmuls can start before
            # the whole working set lands. All on the sync queue: DMA_DIRECT2D
            # occupies the issuing engine's sequencer for the transfer, so
            # the scalar queue must stay clear for ACTIVATEs.
            nc.sync.dma_start(out=sxm_own[:, 0 : ROWS // 2], in_=xmT_own[:, 0 : ROWS // 2])
            nc.sync.dma_start(out=sxm[:, 0:128], in_=xmT_all[:, 0:128])
            nc.sync.dma_start(
                out=sxm_own[:, ROWS // 2 : ROWS], in_=xmT_own[:, ROWS // 2 : ROWS]
            )
            CH = N // 8
            for k in range(8):
                cs = slice(k * CH + (128 if k == 0 else 0), (k + 1) * CH)
                vs = slice(k * (NJT * DY) // 8, (k + 1) * (NJT * DY) // 8)
                nc.sync.dma_start(out=svm[:, vs], in_=vm[:, vs])
                nc.sync.dma_start(out=svv[:, vs], in_=vv[:, vs])
                nc.sync.dma_start(out=sxm[:, cs], in_=xmT_all[:, cs])

            # Z accumulators packed into ONE [128, 1024] PSUM tile (2 banks):
            # mean rows on partitions 0-63, var rows on 64-127 (the matmul
            # writes the var block with tile_position col offset 64). This
            # frees 2 banks so the Gram pool can triple-buffer.
            pz = psz.tile([128, ROWS], f32)
            pzm = pz[0:64, :]
            pzv = pz[64:128, :]

            H = ROWS // 2  # 512-wide halves (PSUM bank / fp32 moving-op limit)

            # Warm-up matmuls on zero data while the first DMAs land: keeps
            # the PE HAM activity window busy so the real matmuls start at
            # 2.4 GHz instead of the cold 1.2 GHz. Overwritten by the real
            # start=True accumulation below.
            warm = data.tile([DM, H], f16)
            nc.vector.memset(warm, 0.0)
            for w in range(2):
                nc.tensor.matmul(
                    (pzm if w % 2 == 0 else pzv)[:, 0:H],
                    warm[:, 0:64], warm[:, :], start=True, stop=True,
                )

            def emit_gram(jt):
                """G'(jt) tile [128 j x ROWS i] into a rotating PSUM slot."""
                jb = slice(jt * 128, (jt + 1) * 128)
                pg = psg.tile([128, ROWS], f32, tag="pgm", bufs=3, name=f"pgm{jt}")
                for h in range(2):
                    hs = slice(h * H, (h + 1) * H)
                    nc.tensor.matmul(
                        pg[:, hs], sxm[:, jb], sxm_own[:, hs], start=True, stop=True
                    )
                return pg

            # Main loop: G'(jt) -> {DVE schraudolph-exp, ACT exact-exp} ->
            # Z accumulation. Double-buffered G slots let the scheduler
            # hoist G(jt+1) matmuls into the activations' shadow.
            grams = [emit_gram(0), emit_gram(1), emit_gram(2)]
            for jt in range(NJT):
                st = jt == 0
                sp = jt == NJT - 1
                pg = grams[jt]

                em = etiles.tile([128, ROWS], f16, tag="em", name=f"em{jt}")
                ev = etiles.tile([128, ROWS], f16, tag="ev", name=f"ev{jt}")
                # The tile dep-tracker chains same-tile readers in emission
                # order (ACT then DVE here); with 3 pg bufs that serial chain
                # spans 3 pipeline periods and stays off the critical path.
                nc.scalar.activation(ev[:, :], pg[:, :], Exp, bias=0.0, scale=KV)
                # em bits = int16(round((G' + B0) max 0)) == fp16 exp(G/49)
                nc.vector.tensor_scalar(em.bitcast(i16), pg[:, :], B0, 0.0, Add, Max)

                vb = slice(jt * DY, (jt + 1) * DY)
                for h in range(2):
                    hs = slice(h * H, (h + 1) * H)
                    nc.tensor.matmul(
                        pzv[:, hs], svv[:, vb], ev[:, hs], start=st, stop=sp
                    )
                    nc.tensor.matmul(
                        pzm[:, hs], svm[:, vb], em[:, hs], start=st, stop=sp
                    )
                if jt + 3 < NJT:
                    grams.append(emit_gram(jt + 3))

            szT = data.tile([128, ROWS], f32)
            nc.scalar.copy(szT[0:64, :], pzm[:, :])
            nc.vector.tensor_copy(szT[64:128, :], pzv[:, :])
            nc.sync.dma_start(out=zT[0:64, :], in_=szT[0:64, :])
            nc.sync.dma_start(out=zT[64:128, :], in_=szT[64:128, :])

    nc.finalize()
    return nc


def _get_nc():
    if "nc" not in _CACHE:
        _CACHE["nc"] = _build_nc()
    return _CACHE["nc"]


def prep_inputs(X_mu, Y_eta, Y_mean, Y_var, V_mean, V_var):
    """Host-side prep: layouts, norms, folded V weights.

    Returns (in_maps, e_m, e_v, ymv)."""
    X_mu, Y_eta, Y_mean, Y_var, V_mean, V_var = (
        np.asarray(a, dtype=np.float32)
        for a in (X_mu, Y_eta, Y_mean, Y_var, V_mean, V_var)
    )
    ymv = (Y_mean.astype(np.float64) + Y_var.astype(np.float64)).astype(np.float32)
    # fp16 features pre-scaled by sqrt(S_SCALE) so the PE emits
    # G' = S_SCALE * G directly (the Schraudolph mantissa scale). fp16
    # products are exact in the fp32 PSUM accumulation, so deriving the
    # row norms from the QUANTIZED features keeps sq consistent.
    sq_s = float(np.sqrt(S_SCALE))
    Xm = np.concatenate([X_mu, ymv], axis=1).astype(np.float32)
    Xq = (Xm * sq_s).astype(np.float16)                              # [N, 96]

    rn = np.sum(Xq.astype(np.float64) ** 2, axis=1)                  # scaled units
    # guard: Schraudolph int16 must stay in (0, 32767):
    # i16 = G' + B0 with |G'| <= max rn
    assert rn.max() + B0 < 32200.0 and B0 - rn.max() > 500.0

    # k = e_i * e_j * exp(G/l^2); exponents in scaled units:
    # G/49 = G'/(1024*log2e), G/81 = G'*KV
    e_m = np.exp(-rn / (2.0 * 1024.0 * LOG2E))                       # fp64 [N]
    e_v = np.exp(-rn * (KV / 2.0))

    xmT = np.ascontiguousarray(Xq.T)                                 # [96, N]
    vm_f = e_m[:, None] * V_mean.astype(np.float64)                  # fold e_j
    vv_f = e_v[:, None] * V_var.astype(np.float64)
    vm_sb = np.ascontiguousarray(
        vm_f.reshape(NJT, 128, DY).transpose(1, 0, 2).reshape(128, NJT * DY)
    ).astype(np.float16)
    vv_sb = np.ascontiguousarray(
        vv_f.reshape(NJT, 128, DY).transpose(1, 0, 2).reshape(128, NJT * DY)
    ).astype(np.float16)

    in_maps = []
    for c in range(NCORES):
        rs = slice(c * ROWS, (c + 1) * ROWS)
        in_maps.append(dict(
            xmT_all=xmT,
            xmT_own=np.ascontiguousarray(Xq[rs].T),
            vm=vm_sb,
            vv=vv_sb,
        ))
    return in_maps, e_m, e_v, ymv


def postprocess(results, e_m, e_v, ymv):
    """Gather per-core z^T outputs and apply the e_i row factors + Y terms."""
    out = ymv.astype(np.float64).copy()
    for c in range(NCORES):
        rs = slice(c * ROWS, (c + 1) * ROWS)
        zT = results[c]["zT"].astype(np.float64)  # [128, ROWS]
        out[rs] += e_m[rs, None] * zT[0:64].T
        out[rs] += e_v[rs, None] * zT[64:128].T
    return out.astype(np.float32)


def kernel(X_mu, Y_eta, Y_mean, Y_var, V_mean, V_var):
    from concourse.bass_utils import run_bass_kernel_spmd

    nc = _get_nc()
    in_maps, e_m, e_v, ymv = prep_inputs(X_mu, Y_eta, Y_mean, Y_var, V_mean, V_var)
    res = run_bass_kernel_spmd(nc, in_maps, core_ids=list(range(NCORES)))
    return postprocess(res.results, e_m, e_v, ymv)
